# revision 56
# baseline (speedup 1.0000x reference)
"""Bass/Trainium2 SPMD kernel for a 2-layer GCN encoder.

Math (per reference):
    src/dst = edges + self-loops
    deg[v]  = #edges with dst==v (incl self-loop);  dinv = 1/sqrt(deg)
    layer(x, W, b): out[d] = dinv[d] * sum_{e: dst_e==d} dinv[src_e] * (x@W)[src_e] + b
    y = layer1(sigmoid(layer0(x, W0, b0)), W1, b1)

Layer 1 is re-associated as (A_n sigmoid(h1)) @ W1 + b1 so that both
aggregation passes gather 128-wide bf16 rows (256B elements).

Distribution: nodes are sharded contiguously across 8 cores (6250 each).
Edges are owned by the destination core.  Each core:
  1. GEMM on its x rows, pre-scales rows by dinv (so the per-edge weight
     dinv[src]*dinv[dst] factorizes into a row pre-scale and an output
     post-scale).  The prescaled rows are AllGathered into two tables:
     table A = every core's local rows [0, sA), table B = the rest.  The
     split lets each AllGather chunk start as soon as its producer blocks
     are done (chunk A moves under the tail of the producing phase), and
     keeps the int16 gather indices in range.
  2. For each 128-row destination block, gathers the source rows of its
     edges (dma_gather on SWDGE queues 0-3 round-robin so descriptor
     generation runs on all four Q7 core pairs concurrently), builds
     one-hot scatter matrices on the vector engine (iota == slot,
     all-bf16), and scatter-adds via TensorE matmuls accumulating in
     PSUM.  Self-loop edges are excluded from the gather and added as an
     identity matmul over the locally-kept prescaled rows.
"""

import math

import numpy as np

import concourse.bacc as bacc
import concourse.bass as bass
import concourse.mybir as mybir
import concourse.tile as tile
from concourse.bass_utils import run_bass_kernel_spmd

P = 128
F32 = mybir.dt.float32
BF16 = mybir.dt.bfloat16
I16 = mybir.dt.int16

# Full-problem constants
N_NODES = 50000
N_CORES = 8
F0, F1, F2 = 128, 128, 64
GROUP_BLOCKS = 2  # dst blocks per dma_gather batch
N_QUEUES = 4  # SWDGE queues (each owns a Q7 core pair)
# Per-(block,half) edge-segment alignment. Must stay 128: sub-128 matmul
# pieces with different base partitions back-to-back hard-crash the PE
# (verified on HW: K64@p0 directly followed by K64@p64 aborts the NEFF).
SEG_ALIGN = 128


class Plan:
    """Compile-time schedule, identical across cores (SPMD)."""

    def __init__(self, n_nodes, n_cores, gb):
        assert n_nodes % n_cores == 0
        self.n_nodes = n_nodes
        self.n_cores = n_cores
        self.npc = n_nodes // n_cores
        self.nblk = math.ceil(self.npc / P)
        # block-aligned per-core row split: rows [0, sA) go to table A,
        # rows [sA, npc) to table B (concatenated across cores)
        self.sA = min(((self.nblk + 1) // 2) * P, self.npc)
        self.sB = self.npc - self.sA
        self.rows = (n_cores * self.sA, n_cores * self.sB)
        assert max(self.rows) <= 32768  # int16 gather indices
        self.gb = gb
        self.groups = [
            list(range(i, min(i + gb, self.nblk))) for i in range(0, self.nblk, gb)
        ]
        self.g_of = {}
        for gi, blocks in enumerate(self.groups):
            for b in blocks:
                self.g_of[b] = gi
        # filled by finalize(): per-(blk, half) uniform padded sizes
        self.SZ = None  # [nblk, 2] int, multiples of SEG_ALIGN
        self.seg_off = {}  # (b, h) -> edge offset within its gather
        self.seg_idx16 = {}  # (g_idx, h) -> int16-column base of that gather
        self.seg_colbase = {}  # (g_idx, h) -> global chunk-column base
        self.gather_nid = {}  # (g_idx, h) -> num idxs
        self.ncols = 0
        self.tot16 = 0

    def finalize(self, sz):
        self.SZ = sz
        col = 0
        i16 = 0
        for gi, blocks in enumerate(self.groups):
            for h in (0, 1):
                off = 0
                for b in blocks:
                    self.seg_off[(b, h)] = off
                    off += int(self.SZ[b, h])
                nid = off
                self.gather_nid[(gi, h)] = nid
                self.seg_idx16[(gi, h)] = i16
                self.seg_colbase[(gi, h)] = col
                col += (nid + P - 1) // P
                i16 += nid // 16
        self.ncols = col
        self.tot16 = i16


def _build_metadata(edges, n_nodes, n_cores, gb=GROUP_BLOCKS):
    """Host-side integer preprocessing: shard + sort edges, build gather
    indices / slot vectors / degree tables.  Returns (plan, per_core dict).

    Self-loops are NOT included in the edge list (their contribution is an
    identity matmul on-device), but they DO count toward the degrees."""
    plan = Plan(n_nodes, n_cores, gb)
    npc, nblk = plan.npc, plan.nblk
    sA, sB = plan.sA, plan.sB

    src = np.asarray(edges[0], dtype=np.int64)
    dst = np.asarray(edges[1], dtype=np.int64)
    deg = (np.bincount(dst, minlength=n_nodes) + 1).astype(np.float32)

    owner = dst // npc
    ldst = dst % npc
    blk = ldst // P
    slot = (ldst % P).astype(np.float32)
    # source's position in the A/B gather tables
    c_src = src // npc
    r_src = src % npc
    half = (r_src >= sA).astype(np.int64)
    tabrow = np.where(half == 0, c_src * sA + r_src, c_src * sB + (r_src - sA))
    cell = ((owner * nblk) + blk) * 2 + half
    order = np.lexsort((tabrow, cell))
    cell_s = cell[order]
    row_s = tabrow[order]
    slot_s = slot[order]

    ncells = n_cores * nblk * 2
    counts = np.bincount(cell_s, minlength=ncells).reshape(n_cores, nblk, 2)
    starts = np.concatenate([[0], np.cumsum(counts.reshape(-1))])[:-1].reshape(
        n_cores, nblk, 2
    )
    sz = np.maximum(counts.max(axis=0), 0)
    sz = (np.ceil(sz / SEG_ALIGN).astype(np.int64)) * SEG_ALIGN  # [nblk, 2]
    plan.finalize(sz)

    ncols = plan.ncols
    tot16 = plan.tot16

    per_core = []
    for c in range(n_cores):
        idx16 = np.zeros((16, tot16), np.int16)
        slots_t = np.full((P, ncols), -1.0, np.float32)
        for gi, blocks in enumerate(plan.groups):
            for h in (0, 1):
                i16b = plan.seg_idx16[(gi, h)] * 16
                colb = plan.seg_colbase[(gi, h)] * P
                for b in blocks:
                    n = int(counts[c, b, h])
                    s0 = int(starts[c, b, h])
                    if n:
                        j = plan.seg_off[(b, h)] + np.arange(n)
                        seg_src = row_s[s0 : s0 + n].astype(np.int16)
                        ji = i16b + j
                        idx16[ji % 16, ji // 16] = seg_src
                        jc = colb + j
                        slots_t[jc % P, jc // P] = slot_s[s0 : s0 + n]
        deg_loc = np.ones(nblk * P, np.float32)
        deg_loc[:npc] = deg[c * npc : (c + 1) * npc]
        dinv_t = 1.0 / np.sqrt(deg_loc.reshape(nblk, P).T.copy())  # [P, nblk]
        per_core.append(
            dict(
                idx16=np.tile(idx16, (8, 1)),  # [128, tot16]
                slots=slots_t,
                dinv=np.ascontiguousarray(dinv_t),
                sqrow=np.sqrt(deg_loc).reshape(1, -1).copy(),
            )
        )
    return plan, per_core


def _build_nc(plan, f0, f1, f2):
    """Build the SPMD bass program (same for every core)."""
    npc, nblk = plan.npc, plan.nblk
    sA, sB = plan.sA, plan.sB
    rows = plan.rows  # rows of each gather table (A, B)
    nc = bacc.Bacc(
        "TRN2",
        target_bir_lowering=False,
        debug=False,
        num_devices=plan.n_cores,
        num_swdge_queues=N_QUEUES,
    )

    # I/O
    xT_d = nc.dram_tensor("xT", [f0, npc], F32, kind="ExternalInput")
    w0_d = nc.dram_tensor("W0", [f0, f1], F32, kind="ExternalInput")
    w1_d = nc.dram_tensor("W1", [f1, f2], F32, kind="ExternalInput")
    b0_d = nc.dram_tensor("b0", [1, f1], BF16, kind="ExternalInput")
    b1_d = nc.dram_tensor("b1", [1, f2], F32, kind="ExternalInput")
    iota_d = nc.dram_tensor("iota", [P, 12 * P], BF16, kind="ExternalInput")
    ident_d = nc.dram_tensor("ident", [P, P], F32, kind="ExternalInput")
    identb_d = nc.dram_tensor("identb", [P, P], BF16, kind="ExternalInput")
    dinv_d = nc.dram_tensor("dinv", [P, nblk], F32, kind="ExternalInput")
    sqrow_d = nc.dram_tensor("sqrow", [1, nblk * P], BF16, kind="ExternalInput")
    idx_d = nc.dram_tensor("idx16", [P, plan.tot16], I16, kind="ExternalInput")
    widx_d = nc.dram_tensor("widx", [P, 8], I16, kind="ExternalInput")
    slots_d = nc.dram_tensor("slots", [P, plan.ncols], BF16, kind="ExternalInput")
    y_d = nc.dram_tensor("y", [npc, f2], F32, kind="ExternalOutput")

    rg = [list(range(plan.n_cores))]
    AF = mybir.ActivationFunctionType
    nblkA = sA // P  # blocks feeding table A

    def ag(loc, full):
        nc.gpsimd.collective_compute(
            "AllGather",
            mybir.AluOpType.bypass,
            replica_groups=rg,
            ins=[loc[:, :].opt()],
            outs=[full[:, :].opt()],
        )

    with tile.TileContext(nc) as tc:
        with (
            tc.tile_pool(name="dram", bufs=1, space="DRAM") as dramp,
            tc.tile_pool(name="const", bufs=1) as constp,
            tc.tile_pool(name="gath", bufs=16) as gpool,
            tc.tile_pool(name="sel", bufs=4) as spool,
            tc.tile_pool(name="stage", bufs=4) as stpool,
            tc.tile_pool(name="pgemm", bufs=2, space="PSUM") as pgemm,
            tc.tile_pool(name="pscat", bufs=2, space="PSUM") as pscat,
            tc.tile_pool(name="ptrans", bufs=2, space="PSUM") as ptrans,
        ):
            h1_locA = dramp.tile([sA, f1], BF16, name="h1_locA")
            h1_locB = dramp.tile([sB, f1], BF16, name="h1_locB")
            h1_fullA = dramp.tile([rows[0], f1], BF16, addr_space="Shared",
                                  name="h1_fullA")
            h1_fullB = dramp.tile([rows[1], f1], BF16, addr_space="Shared",
                                  name="h1_fullB")
            h2_locA = dramp.tile([sA, f1], BF16, name="h2_locA")
            h2_locB = dramp.tile([sB, f1], BF16, name="h2_locB")
            h2_fullA = dramp.tile([rows[0], f1], BF16, addr_space="Shared",
                                  name="h2_fullA")
            h2_fullB = dramp.tile([rows[1], f1], BF16, addr_space="Shared",
                                  name="h2_fullB")
            ccw_loc = dramp.tile([1, P], F32, name="ccw_loc")
            ccw_full = dramp.tile([plan.n_cores, P], F32, addr_space="Shared",
                                  name="ccw_full")

            # warm the CC stream with a tiny AllGather FIRST (staged on the
            # otherwise-empty scalar queue) so the ~60-80us collective
            # startup latency overlaps the GEMM head instead of AG0
            ones_t = constp.tile([1, P], F32, name="ones_t")
            nc.vector.memset(ones_t[:], 1.0)
            nc.scalar.dma_start(out=ccw_loc[:, :], in_=ones_t[:])
            ag(ccw_loc, ccw_full)

            # ---- constants / metadata ----
            def load_const(name, dram, shape, dtype=F32):
                t = constp.tile(shape, dtype, name=name)
                nc.sync.dma_start(out=t[:], in_=dram[:])
                return t

            # ordered so the L0 GEMM -> AllGather chain starts ASAP; the big
            # gather metadata loads overlap with it
            xT_t = load_const("xT_t", xT_d, [f0, npc])
            w0_t = load_const("w0_t", w0_d, [f0, f1])
            dinv_t = load_const("dinv_t", dinv_d, [P, nblk])
            w1_t = load_const("w1_t", w1_d, [f1, f2])
            b0_t = load_const("b0_t", b0_d, [1, f1], BF16)
            b1_t = load_const("b1_t", b1_d, [1, f2])
            iota_t = load_const("iota_t", iota_d, [P, 12 * P], BF16)
            ident_t = load_const("ident_t", ident_d, [P, P])
            identb_t = load_const("identb_t", identb_d, [P, P], BF16)
            sqrow_t = load_const("sqrow_t", sqrow_d, [1, nblk * P], BF16)
            widx_t = load_const("widx_t", widx_d, [P, 8], I16)
            idx_t = load_const("idx_t", idx_d, [P, plan.tot16], I16)
            slots_t = load_const("slots_t", slots_d, [P, plan.ncols], BF16)

            # per-layer local prescaled rows (self-loop term), bf16
            hpre0_t = constp.tile([P, nblk * f1], BF16, name="hpre0_t")
            hpre1_t = constp.tile([P, nblk * f1], BF16, name="hpre1_t")
            # zero the last (partial) block's columns so the identity matmul
            # never streams garbage (NaN) through the PE; the GEMM/sigmoid
            # writers later fill rows [0, last_wb)
            nc.vector.memset(hpre0_t[:, (nblk - 1) * f1 :], 0.0)
            nc.vector.memset(hpre1_t[:, (nblk - 1) * f1 :], 0.0)

            # warm the Q7 dma_gather ucode on every queue pair (first gather
            # per pair pays the icache fill; do it under the GEMM+AllGather
            # head instead)
            for q in range(N_QUEUES):
                warm_t = constp.tile([P, 1, 64], F32, name=f"warm{q}_t")
                nc.gpsimd.dma_gather(
                    warm_t[:],
                    ident_d[:, 0:64],
                    widx_t[:, 0:8],
                    128,
                    128,
                    64,
                    elem_step=P,
                    single_packet=False,
                    queue_num=q,
                )

            def loc_dma(locA, locB, hs, b, wb):
                """DMA block b's rows of hs into the A/B local shard tiles."""
                if b < nblkA:
                    nc.sync.dma_start(
                        out=locA[b * P : b * P + wb, :], in_=hs[:wb, :]
                    )
                else:
                    r0 = b * P - sA
                    nc.sync.dma_start(out=locB[r0 : r0 + wb, :], in_=hs[:wb, :])

            def gemm_layer(src_sbuf, w_t, fout, hpre_t):
                """h1_loc rows = hpre rows = dinv * (x @ W), bf16."""
                for t in range(nblk):
                    wt = min(P, npc - t * P)
                    hp = pgemm.tile([P, fout], F32, name="hp")
                    nc.tensor.matmul(
                        hp[:wt, :],
                        src_sbuf[:, t * P : t * P + wt],
                        w_t[:],
                        start=True,
                        stop=True,
                    )
                    hs = hpre_t[:, t * fout : (t + 1) * fout]
                    nc.scalar.activation(
                        hs[:wt, :],
                        hp[:wt, :],
                        AF.Copy,
                        scale=dinv_t[:wt, t : t + 1],
                    )
                    loc_dma(h1_locA, h1_locB, hs, t, wt)
                    if t == nblkA - 1:
                        ag(h1_locA, h1_fullA)  # chunk A under the GEMM tail
                ag(h1_locB, h1_fullB)

            qctr = [0]

            def scatter_layer(h_fullA, h_fullB, bias_t, hpre_t, hpre_out,
                              is_last):
                """For every dst block: gather + one-hot matmul scatter-add.

                is_last=False (layer 0): pb = sqdeg*b0 + self + edges;
                  writes h2_loc rows = dinv*sigmoid(dinv*pb) and hpre1.
                is_last=True (layer 1): pb = self + edges (A_n-aggregation
                  of sigmoid rows); then y = (dinv*pb) @ W1 + b1 via a
                  PE transpose + GEMM with a rank-1 bias matmul."""
                fout = f1
                h_full = (h_fullA, h_fullB)
                g_of = plan.g_of
                gts = {}

                def emit_gather(gi, h):
                    nid = plan.gather_nid[(gi, h)]
                    if nid == 0:
                        return
                    ncol = (nid + P - 1) // P
                    g_tile = gpool.tile(
                        [P, ncol, fout], BF16, tag="gath", name=f"g{gi}_{h}"
                    )
                    i0 = plan.seg_idx16[(gi, h)]
                    nc.gpsimd.dma_gather(
                        g_tile[:],
                        h_full[h][:, :],
                        idx_t[:, i0 : i0 + nid // 16],
                        nid,
                        nid,
                        fout,
                        single_packet=False,
                        queue_num=qctr[0] % N_QUEUES,
                    )
                    qctr[0] += 1
                    gts[(gi, h)] = g_tile

                # table-A gathers of the first groups go first: their
                # descriptor generation runs while AllGather chunk B is
                # still in flight (table A lands first)
                for gi in range(min(8, len(plan.groups))):
                    emit_gather(gi, 0)
                for gi, blocks in enumerate(plan.groups):
                    for h in (0, 1):
                        if (gi, h) not in gts:
                            emit_gather(gi, h)
                    gt = {h: gts.pop((gi, h)) for h in (0, 1)
                          if (gi, h) in gts}
                    for b in blocks:
                        wb = min(P, npc - b * P)
                        pb = pscat.tile([P, fout], F32, name="pb")
                        if is_last:
                            # self-loop term opens the accumulation
                            nc.tensor.matmul(
                                pb[:],
                                identb_t[:],
                                hpre_t[:, b * f1 : (b + 1) * f1],
                                start=True,
                                stop=False,
                            )
                        else:
                            nc.tensor.matmul(
                                pb[:],
                                sqrow_t[0:1, b * P : (b + 1) * P],
                                bias_t[:],
                                start=True,
                                stop=False,
                            )
                            nc.tensor.matmul(
                                pb[:],
                                identb_t[:],
                                hpre_t[:, b * f1 : (b + 1) * f1],
                                start=False,
                                stop=False,
                            )
                        pieces = []  # (h, col, p0, p1)
                        sels = {}
                        spans = {}  # h -> (first_col, ncols)
                        for h in (0, 1):
                            sz = int(plan.SZ[b, h])
                            if sz == 0:
                                continue
                            off = plan.seg_off[(b, h)]
                            c_lo = off // P
                            c_hi = (off + sz - 1) // P
                            spans[h] = (c_lo, c_hi - c_lo + 1)
                            for c in range(c_lo, c_hi + 1):
                                p0 = max(0, off - P * c)
                                p1 = min(P, off + sz - P * c)
                                pieces.append((h, c, p0, p1))
                        for h, (c_lo, nch) in spans.items():
                            colb = plan.seg_colbase[(g_of[b], h)]
                            sel = spool.tile(
                                [P, nch, P], BF16, tag="sel", name="sel"
                            )
                            nc.vector.tensor_tensor(
                                out=sel[:],
                                in0=slots_t[
                                    :, colb + c_lo : colb + c_lo + nch
                                ].to_broadcast([P, nch, P]),
                                in1=iota_t[:, 0 : nch * P].rearrange(
                                    "p (a b) -> p a b", b=P
                                ),
                                op=mybir.AluOpType.is_equal,
                            )
                            sels[h] = (sel, c_lo)
                        for k, (h, c, p0, p1) in enumerate(pieces):
                            sel, c_lo = sels[h]
                            nc.tensor.matmul(
                                pb[:],
                                sel[p0:p1, c - c_lo, :],
                                gt[h][p0:p1, c, :],
                                start=False,
                                stop=(k == len(pieces) - 1),
                            )
                        if is_last:
                            # agg = dinv * pb  (A_n-aggregated sigmoid rows)
                            ob = stpool.tile([P, fout], F32, tag="ob", name="ob")
                            nc.scalar.activation(
                                ob[:],
                                pb[:],
                                AF.Copy,
                                scale=dinv_t[:, b : b + 1],
                            )
                            # y[block] = agg @ W1 + b1
                            pt = ptrans.tile([P, P], F32, name="pt")
                            nc.tensor.transpose(pt[:], ob[:], ident_t[:])
                            aggT = stpool.tile([P, P], F32, tag="aggT",
                                               name="aggT")
                            nc.vector.tensor_copy(aggT[:], pt[:])
                            yp = pgemm.tile([P, f2], F32, name="yp")
                            nc.tensor.matmul(
                                yp[:wb, :],
                                aggT[:, :wb],
                                w1_t[:],
                                start=True,
                                stop=False,
                            )
                            nc.tensor.matmul(
                                yp[:wb, :],
                                ones_t[0:1, :wb],
                                b1_t[:],
                                start=False,
                                stop=True,
                            )
                            ys = stpool.tile([P, f2], F32, tag="ys", name="ys")
                            nc.vector.tensor_copy(ys[:wb, :], yp[:wb, :])
                            nc.sync.dma_start(
                                out=y_d[b * P : b * P + wb, :], in_=ys[:wb, :]
                            )
                        else:
                            # h2 rows = dinv * sigmoid(dinv * pb), bf16; also
                            # kept in hpre1 as the layer-1 self-loop term
                            sg = stpool.tile([P, fout], F32, tag="sg", name="sg")
                            nc.scalar.activation(
                                sg[:],
                                pb[:],
                                AF.Sigmoid,
                                scale=dinv_t[:, b : b + 1],
                            )
                            hs2 = hpre_out[:, b * f1 : (b + 1) * f1]
                            nc.vector.tensor_tensor(
                                out=hs2,
                                in0=sg[:],
                                in1=dinv_t[:, b : b + 1].to_broadcast([P, f1]),
                                op=mybir.AluOpType.mult,
                            )
                            loc_dma(h2_locA, h2_locB, hs2, b, wb)
                            if b == nblkA - 1:
                                # chunk A moves under the rest of the scatter
                                ag(h2_locA, h2_fullA)

            # ---- layer 0 ----
            gemm_layer(xT_t, w0_t, f1, hpre0_t)
            scatter_layer(h1_fullA, h1_fullB, b0_t, hpre0_t, hpre1_t,
                          is_last=False)

            # ---- layer 1 ----
            ag(h2_locB, h2_fullB)  # chunk A was emitted mid-scatter
            scatter_layer(h2_fullA, h2_fullB, b1_t, hpre1_t, None,
                          is_last=True)

    nc.compile()
    return nc


def _make_in_maps(x, W0, b0, W1, b1, plan, per_core):
    import ml_dtypes

    npc = plan.npc
    x = np.asarray(x, dtype=np.float32)
    shared = dict(
        W0=np.asarray(W0, np.float32).reshape(W0.shape[0], -1),
        W1=np.asarray(W1, np.float32).reshape(W1.shape[0], -1),
        b0=np.asarray(b0, np.float32).reshape(1, -1).astype(ml_dtypes.bfloat16),
        b1=np.asarray(b1, np.float32).reshape(1, -1),
        iota=np.tile(
            np.arange(P, dtype=np.float32)[None, :], (P, 12)
        ).astype(ml_dtypes.bfloat16),
        ident=np.eye(P, dtype=np.float32),
        identb=np.eye(P, dtype=np.float32).astype(ml_dtypes.bfloat16),
    )
    in_maps = []
    for c in range(plan.n_cores):
        m = dict(shared)
        m["xT"] = np.ascontiguousarray(x[c * npc : (c + 1) * npc, :].T)
        m["idx16"] = per_core[c]["idx16"]
        m["widx"] = np.zeros((P, 8), np.int16)
        m["slots"] = per_core[c]["slots"].astype(ml_dtypes.bfloat16)
        m["dinv"] = per_core[c]["dinv"]
        m["sqrow"] = per_core[c]["sqrow"].astype(ml_dtypes.bfloat16)
        in_maps.append(m)
    return in_maps


_CACHE = {}


def build(x, edges, W0, b0, W1, b1, n_nodes=N_NODES, n_cores=N_CORES,
          gb=GROUP_BLOCKS):
    """Returns (nc, in_maps, plan). Cached on the edge structure size."""
    plan, per_core = _build_metadata(edges, n_nodes, n_cores, gb)
    key = (n_nodes, n_cores, gb, tuple(plan.SZ.reshape(-1).tolist()))
    if key not in _CACHE:
        _CACHE[key] = _build_nc(plan, x.shape[1], W0.shape[1], W1.shape[1])
    nc = _CACHE[key]
    in_maps = _make_in_maps(x, W0, b0, W1, b1, plan, per_core)
    return nc, in_maps, plan


def kernel(x, edges, W0, b0, W1, b1):
    x = np.asarray(x)
    nc, in_maps, plan = build(x, edges, W0, b0, W1, b1)
    res = run_bass_kernel_spmd(nc, in_maps, list(range(plan.n_cores)))
    y = np.concatenate([r["y"] for r in res.results], axis=0)
    return y.astype(np.float32)


# revision 57
# speedup vs baseline: 1.0119x; 1.0119x over previous
"""Bass/Trainium2 SPMD kernel for a 2-layer GCN encoder.

Math (per reference):
    src/dst = edges + self-loops
    deg[v]  = #edges with dst==v (incl self-loop);  dinv = 1/sqrt(deg)
    layer(x, W, b): out[d] = dinv[d] * sum_{e: dst_e==d} dinv[src_e] * (x@W)[src_e] + b
    y = layer1(sigmoid(layer0(x, W0, b0)), W1, b1)

Layer 1 is re-associated as (A_n sigmoid(h1)) @ W1 + b1 so that both
aggregation passes gather 128-wide bf16 rows (256B elements).

Distribution: nodes are sharded contiguously across 8 cores (6250 each).
Edges are owned by the destination core.  Each core:
  1. GEMM on its x rows, pre-scales rows by dinv (so the per-edge weight
     dinv[src]*dinv[dst] factorizes into a row pre-scale and an output
     post-scale).  The prescaled rows are AllGathered into two tables:
     table A = every core's local rows [0, sA), table B = the rest.  The
     split lets each AllGather chunk start as soon as its producer blocks
     are done (chunk A moves under the tail of the producing phase), and
     keeps the int16 gather indices in range.
  2. For each 128-row destination block, gathers the source rows of its
     edges (dma_gather on SWDGE queues 0-3 round-robin so descriptor
     generation runs on all four Q7 core pairs concurrently), builds
     one-hot scatter matrices on the vector engine (iota == slot,
     all-bf16), and scatter-adds via TensorE matmuls accumulating in
     PSUM.  Self-loop edges are excluded from the gather and added as an
     identity matmul over the locally-kept prescaled rows.
"""

import math

import numpy as np

import concourse.bacc as bacc
import concourse.bass as bass
import concourse.mybir as mybir
import concourse.tile as tile
from concourse.bass_utils import run_bass_kernel_spmd

P = 128
F32 = mybir.dt.float32
BF16 = mybir.dt.bfloat16
I16 = mybir.dt.int16

# Full-problem constants
N_NODES = 50000
N_CORES = 8
F0, F1, F2 = 128, 128, 64
GROUP_BLOCKS = 2  # dst blocks per dma_gather batch
N_QUEUES = 4  # SWDGE queues (each owns a Q7 core pair)
# Per-(block,half) edge-segment alignment. Must stay 128: sub-128 matmul
# pieces with different base partitions back-to-back hard-crash the PE
# (verified on HW: K64@p0 directly followed by K64@p64 aborts the NEFF).
SEG_ALIGN = 128


class Plan:
    """Compile-time schedule, identical across cores (SPMD)."""

    def __init__(self, n_nodes, n_cores, gb):
        assert n_nodes % n_cores == 0
        self.n_nodes = n_nodes
        self.n_cores = n_cores
        self.npc = n_nodes // n_cores
        self.nblk = math.ceil(self.npc / P)
        # block-aligned per-core row split: rows [0, sA) go to table A,
        # rows [sA, npc) to table B (concatenated across cores)
        self.sA = min(((self.nblk + 1) // 2) * P, self.npc)
        self.sB = self.npc - self.sA
        self.rows = (n_cores * self.sA, n_cores * self.sB)
        assert max(self.rows) <= 32768  # int16 gather indices
        self.gb = gb
        self.groups = [
            list(range(i, min(i + gb, self.nblk))) for i in range(0, self.nblk, gb)
        ]
        self.g_of = {}
        for gi, blocks in enumerate(self.groups):
            for b in blocks:
                self.g_of[b] = gi
        # filled by finalize(): per-(blk, half) uniform padded sizes
        self.SZ = None  # [nblk, 2] int, multiples of SEG_ALIGN
        self.seg_off = {}  # (b, h) -> edge offset within its gather
        self.seg_idx16 = {}  # (g_idx, h) -> int16-column base of that gather
        self.seg_colbase = {}  # (g_idx, h) -> global chunk-column base
        self.gather_nid = {}  # (g_idx, h) -> num idxs
        self.ncols = 0
        self.tot16 = 0

    def finalize(self, sz):
        self.SZ = sz
        col = 0
        i16 = 0
        for gi, blocks in enumerate(self.groups):
            for h in (0, 1):
                off = 0
                for b in blocks:
                    self.seg_off[(b, h)] = off
                    off += int(self.SZ[b, h])
                nid = off
                self.gather_nid[(gi, h)] = nid
                self.seg_idx16[(gi, h)] = i16
                self.seg_colbase[(gi, h)] = col
                col += (nid + P - 1) // P
                i16 += nid // 16
        self.ncols = col
        self.tot16 = i16


def _build_metadata(edges, n_nodes, n_cores, gb=GROUP_BLOCKS):
    """Host-side integer preprocessing: shard + sort edges, build gather
    indices / slot vectors / degree tables.  Returns (plan, per_core dict).

    Self-loops are NOT included in the edge list (their contribution is an
    identity matmul on-device), but they DO count toward the degrees."""
    plan = Plan(n_nodes, n_cores, gb)
    npc, nblk = plan.npc, plan.nblk
    sA, sB = plan.sA, plan.sB

    src = np.asarray(edges[0], dtype=np.int64)
    dst = np.asarray(edges[1], dtype=np.int64)
    deg = (np.bincount(dst, minlength=n_nodes) + 1).astype(np.float32)

    owner = dst // npc
    ldst = dst % npc
    blk = ldst // P
    slot = (ldst % P).astype(np.float32)
    # source's position in the A/B gather tables
    c_src = src // npc
    r_src = src % npc
    half = (r_src >= sA).astype(np.int64)
    tabrow = np.where(half == 0, c_src * sA + r_src, c_src * sB + (r_src - sA))
    cell = ((owner * nblk) + blk) * 2 + half
    order = np.lexsort((tabrow, cell))
    cell_s = cell[order]
    row_s = tabrow[order]
    slot_s = slot[order]

    ncells = n_cores * nblk * 2
    counts = np.bincount(cell_s, minlength=ncells).reshape(n_cores, nblk, 2)
    starts = np.concatenate([[0], np.cumsum(counts.reshape(-1))])[:-1].reshape(
        n_cores, nblk, 2
    )
    sz = np.maximum(counts.max(axis=0), 0)
    sz = (np.ceil(sz / SEG_ALIGN).astype(np.int64)) * SEG_ALIGN  # [nblk, 2]
    plan.finalize(sz)

    ncols = plan.ncols
    tot16 = plan.tot16

    per_core = []
    for c in range(n_cores):
        idx16 = np.zeros((16, tot16), np.int16)
        slots_t = np.full((P, ncols), -1.0, np.float32)
        for gi, blocks in enumerate(plan.groups):
            for h in (0, 1):
                i16b = plan.seg_idx16[(gi, h)] * 16
                colb = plan.seg_colbase[(gi, h)] * P
                for b in blocks:
                    n = int(counts[c, b, h])
                    s0 = int(starts[c, b, h])
                    if n:
                        j = plan.seg_off[(b, h)] + np.arange(n)
                        seg_src = row_s[s0 : s0 + n].astype(np.int16)
                        ji = i16b + j
                        idx16[ji % 16, ji // 16] = seg_src
                        jc = colb + j
                        slots_t[jc % P, jc // P] = slot_s[s0 : s0 + n]
        deg_loc = np.ones(nblk * P, np.float32)
        deg_loc[:npc] = deg[c * npc : (c + 1) * npc]
        dinv_t = 1.0 / np.sqrt(deg_loc.reshape(nblk, P).T.copy())  # [P, nblk]
        per_core.append(
            dict(
                idx16=np.tile(idx16, (8, 1)),  # [128, tot16]
                slots=slots_t,
                dinv=np.ascontiguousarray(dinv_t),
                sqrow=np.sqrt(deg_loc).reshape(1, -1).copy(),
            )
        )
    return plan, per_core


def _build_nc(plan, f0, f1, f2):
    """Build the SPMD bass program (same for every core)."""
    npc, nblk = plan.npc, plan.nblk
    sA, sB = plan.sA, plan.sB
    rows = plan.rows  # rows of each gather table (A, B)
    nc = bacc.Bacc(
        "TRN2",
        target_bir_lowering=False,
        debug=False,
        num_devices=plan.n_cores,
        num_swdge_queues=N_QUEUES,
    )

    # I/O
    xT_d = nc.dram_tensor("xT", [f0, npc], F32, kind="ExternalInput")
    w0_d = nc.dram_tensor("W0", [f0, f1], F32, kind="ExternalInput")
    w1_d = nc.dram_tensor("W1", [f1, f2], F32, kind="ExternalInput")
    b0_d = nc.dram_tensor("b0", [1, f1], BF16, kind="ExternalInput")
    b1_d = nc.dram_tensor("b1", [1, f2], F32, kind="ExternalInput")
    iota_d = nc.dram_tensor("iota", [P, 12 * P], BF16, kind="ExternalInput")
    ident_d = nc.dram_tensor("ident", [P, P], F32, kind="ExternalInput")
    identb_d = nc.dram_tensor("identb", [P, P], BF16, kind="ExternalInput")
    dinv_d = nc.dram_tensor("dinv", [P, nblk], F32, kind="ExternalInput")
    sqrow_d = nc.dram_tensor("sqrow", [1, nblk * P], BF16, kind="ExternalInput")
    idx_d = nc.dram_tensor("idx16", [P, plan.tot16], I16, kind="ExternalInput")
    widx_d = nc.dram_tensor("widx", [P, 8], I16, kind="ExternalInput")
    slots_d = nc.dram_tensor("slots", [P, plan.ncols], BF16, kind="ExternalInput")
    y_d = nc.dram_tensor("y", [npc, f2], F32, kind="ExternalOutput")

    rg = [list(range(plan.n_cores))]
    AF = mybir.ActivationFunctionType
    nblkA = sA // P  # blocks feeding table A

    def ag(loc, full):
        nc.gpsimd.collective_compute(
            "AllGather",
            mybir.AluOpType.bypass,
            replica_groups=rg,
            ins=[loc[:, :].opt()],
            outs=[full[:, :].opt()],
        )

    with tile.TileContext(nc) as tc:
        with (
            tc.tile_pool(name="dram", bufs=1, space="DRAM") as dramp,
            tc.tile_pool(name="const", bufs=1) as constp,
            tc.tile_pool(name="gath", bufs=16) as gpool,
            tc.tile_pool(name="sel", bufs=4) as spool,
            tc.tile_pool(name="stage", bufs=4) as stpool,
            tc.tile_pool(name="pgemm", bufs=2, space="PSUM") as pgemm,
            tc.tile_pool(name="pscat", bufs=2, space="PSUM") as pscat,
            tc.tile_pool(name="ptrans", bufs=2, space="PSUM") as ptrans,
        ):
            h1_locA = dramp.tile([sA, f1], BF16, name="h1_locA")
            h1_locB = dramp.tile([sB, f1], BF16, name="h1_locB")
            h1_fullA = dramp.tile([rows[0], f1], BF16, addr_space="Shared",
                                  name="h1_fullA")
            h1_fullB = dramp.tile([rows[1], f1], BF16, addr_space="Shared",
                                  name="h1_fullB")
            h2_locA = dramp.tile([sA, f1], BF16, name="h2_locA")
            h2_locB = dramp.tile([sB, f1], BF16, name="h2_locB")
            h2_fullA = dramp.tile([rows[0], f1], BF16, addr_space="Shared",
                                  name="h2_fullA")
            h2_fullB = dramp.tile([rows[1], f1], BF16, addr_space="Shared",
                                  name="h2_fullB")
            ccw_loc = dramp.tile([1, P], F32, name="ccw_loc")
            ccw_full = dramp.tile([plan.n_cores, P], F32, addr_space="Shared",
                                  name="ccw_full")

            # warm the CC stream with a tiny AllGather FIRST (staged on the
            # otherwise-empty scalar queue) so the ~60-80us collective
            # startup latency overlaps the GEMM head instead of AG0
            ones_t = constp.tile([1, P], F32, name="ones_t")
            nc.vector.memset(ones_t[:], 1.0)
            nc.scalar.dma_start(out=ccw_loc[:, :], in_=ones_t[:])
            ag(ccw_loc, ccw_full)

            # ---- constants / metadata ----
            def load_const(name, dram, shape, dtype=F32):
                t = constp.tile(shape, dtype, name=name)
                nc.sync.dma_start(out=t[:], in_=dram[:])
                return t

            # ordered so the L0 GEMM -> AllGather chain starts ASAP; the big
            # gather metadata loads overlap with it
            xT_t = load_const("xT_t", xT_d, [f0, npc])
            w0_t = load_const("w0_t", w0_d, [f0, f1])
            dinv_t = load_const("dinv_t", dinv_d, [P, nblk])
            w1_t = load_const("w1_t", w1_d, [f1, f2])
            b0_t = load_const("b0_t", b0_d, [1, f1], BF16)
            b1_t = load_const("b1_t", b1_d, [1, f2])
            iota_t = load_const("iota_t", iota_d, [P, 12 * P], BF16)
            ident_t = load_const("ident_t", ident_d, [P, P])
            identb_t = load_const("identb_t", identb_d, [P, P], BF16)
            sqrow_t = load_const("sqrow_t", sqrow_d, [1, nblk * P], BF16)
            widx_t = load_const("widx_t", widx_d, [P, 8], I16)
            idx_t = load_const("idx_t", idx_d, [P, plan.tot16], I16)
            slots_t = load_const("slots_t", slots_d, [P, plan.ncols], BF16)

            # per-layer local prescaled rows (self-loop term), bf16
            hpre0_t = constp.tile([P, nblk * f1], BF16, name="hpre0_t")
            hpre1_t = constp.tile([P, nblk * f1], BF16, name="hpre1_t")
            # zero the last (partial) block's columns so the identity matmul
            # never streams garbage (NaN) through the PE; the GEMM/sigmoid
            # writers later fill rows [0, last_wb)
            nc.vector.memset(hpre0_t[:, (nblk - 1) * f1 :], 0.0)
            nc.vector.memset(hpre1_t[:, (nblk - 1) * f1 :], 0.0)

            # warm the Q7 dma_gather ucode on every queue pair (first gather
            # per pair pays the icache fill; do it under the GEMM+AllGather
            # head instead)
            for q in range(N_QUEUES):
                warm_t = constp.tile([P, 1, 64], F32, name=f"warm{q}_t")
                nc.gpsimd.dma_gather(
                    warm_t[:],
                    ident_d[:, 0:64],
                    widx_t[:, 0:8],
                    128,
                    128,
                    64,
                    elem_step=P,
                    single_packet=False,
                    queue_num=q,
                )

            def loc_dma(locA, locB, hs, b, wb):
                """DMA block b's rows of hs into the A/B local shard tiles."""
                if b < nblkA:
                    nc.sync.dma_start(
                        out=locA[b * P : b * P + wb, :], in_=hs[:wb, :]
                    )
                else:
                    r0 = b * P - sA
                    nc.sync.dma_start(out=locB[r0 : r0 + wb, :], in_=hs[:wb, :])

            def gemm_layer(src_sbuf, w_t, fout, hpre_t):
                """h1_loc rows = hpre rows = dinv * (x @ W), bf16."""
                for t in range(nblk):
                    wt = min(P, npc - t * P)
                    hp = pgemm.tile([P, fout], F32, name="hp")
                    nc.tensor.matmul(
                        hp[:wt, :],
                        src_sbuf[:, t * P : t * P + wt],
                        w_t[:],
                        start=True,
                        stop=True,
                    )
                    hs = hpre_t[:, t * fout : (t + 1) * fout]
                    nc.scalar.activation(
                        hs[:wt, :],
                        hp[:wt, :],
                        AF.Copy,
                        scale=dinv_t[:wt, t : t + 1],
                    )
                    loc_dma(h1_locA, h1_locB, hs, t, wt)
                    if t == nblkA - 1:
                        ag(h1_locA, h1_fullA)  # chunk A under the GEMM tail
                ag(h1_locB, h1_fullB)

            qctr = [0]

            def scatter_layer(h_fullA, h_fullB, bias_t, hpre_t, hpre_out,
                              is_last):
                """For every dst block: gather + one-hot matmul scatter-add.

                is_last=False (layer 0): pb = sqdeg*b0 + self + edges;
                  writes h2_loc rows = dinv*sigmoid(dinv*pb) and hpre1.
                is_last=True (layer 1): pb = self + edges (A_n-aggregation
                  of sigmoid rows); then y = (dinv*pb) @ W1 + b1 via a
                  PE transpose + GEMM with a rank-1 bias matmul."""
                fout = f1
                h_full = (h_fullA, h_fullB)
                g_of = plan.g_of
                gts = {}

                def emit_gather(gi, h):
                    nid = plan.gather_nid[(gi, h)]
                    if nid == 0:
                        return
                    ncol = (nid + P - 1) // P
                    g_tile = gpool.tile(
                        [P, ncol, fout], BF16, tag="gath", name=f"g{gi}_{h}"
                    )
                    i0 = plan.seg_idx16[(gi, h)]
                    nc.gpsimd.dma_gather(
                        g_tile[:],
                        h_full[h][:, :],
                        idx_t[:, i0 : i0 + nid // 16],
                        nid,
                        nid,
                        fout,
                        single_packet=False,
                        queue_num=qctr[0] % N_QUEUES,
                    )
                    qctr[0] += 1
                    gts[(gi, h)] = g_tile

                # table-A gathers of the first groups go first: their
                # descriptor generation runs while AllGather chunk B is
                # still in flight (table A lands first)
                for gi in range(min(4, len(plan.groups))):
                    emit_gather(gi, 0)
                for gi, blocks in enumerate(plan.groups):
                    for h in (0, 1):
                        if (gi, h) not in gts:
                            emit_gather(gi, h)
                    gt = {h: gts.pop((gi, h)) for h in (0, 1)
                          if (gi, h) in gts}
                    for b in blocks:
                        wb = min(P, npc - b * P)
                        pb = pscat.tile([P, fout], F32, name="pb")
                        if is_last:
                            # self-loop term opens the accumulation
                            nc.tensor.matmul(
                                pb[:],
                                identb_t[:],
                                hpre_t[:, b * f1 : (b + 1) * f1],
                                start=True,
                                stop=False,
                            )
                        else:
                            nc.tensor.matmul(
                                pb[:],
                                sqrow_t[0:1, b * P : (b + 1) * P],
                                bias_t[:],
                                start=True,
                                stop=False,
                            )
                            nc.tensor.matmul(
                                pb[:],
                                identb_t[:],
                                hpre_t[:, b * f1 : (b + 1) * f1],
                                start=False,
                                stop=False,
                            )
                        pieces = []  # (h, col, p0, p1)
                        sels = {}
                        spans = {}  # h -> (first_col, ncols)
                        for h in (0, 1):
                            sz = int(plan.SZ[b, h])
                            if sz == 0:
                                continue
                            off = plan.seg_off[(b, h)]
                            c_lo = off // P
                            c_hi = (off + sz - 1) // P
                            spans[h] = (c_lo, c_hi - c_lo + 1)
                            for c in range(c_lo, c_hi + 1):
                                p0 = max(0, off - P * c)
                                p1 = min(P, off + sz - P * c)
                                pieces.append((h, c, p0, p1))
                        for h, (c_lo, nch) in spans.items():
                            colb = plan.seg_colbase[(g_of[b], h)]
                            sel = spool.tile(
                                [P, nch, P], BF16, tag="sel", name="sel"
                            )
                            nc.vector.tensor_tensor(
                                out=sel[:],
                                in0=slots_t[
                                    :, colb + c_lo : colb + c_lo + nch
                                ].to_broadcast([P, nch, P]),
                                in1=iota_t[:, 0 : nch * P].rearrange(
                                    "p (a b) -> p a b", b=P
                                ),
                                op=mybir.AluOpType.is_equal,
                            )
                            sels[h] = (sel, c_lo)
                        for k, (h, c, p0, p1) in enumerate(pieces):
                            sel, c_lo = sels[h]
                            nc.tensor.matmul(
                                pb[:],
                                sel[p0:p1, c - c_lo, :],
                                gt[h][p0:p1, c, :],
                                start=False,
                                stop=(k == len(pieces) - 1),
                            )
                        if is_last:
                            # agg = dinv * pb  (A_n-aggregated sigmoid rows)
                            ob = stpool.tile([P, fout], F32, tag="ob", name="ob")
                            nc.scalar.activation(
                                ob[:],
                                pb[:],
                                AF.Copy,
                                scale=dinv_t[:, b : b + 1],
                            )
                            # y[block] = agg @ W1 + b1
                            pt = ptrans.tile([P, P], F32, name="pt")
                            nc.tensor.transpose(pt[:], ob[:], ident_t[:])
                            aggT = stpool.tile([P, P], F32, tag="aggT",
                                               name="aggT")
                            nc.vector.tensor_copy(aggT[:], pt[:])
                            yp = pgemm.tile([P, f2], F32, name="yp")
                            nc.tensor.matmul(
                                yp[:wb, :],
                                aggT[:, :wb],
                                w1_t[:],
                                start=True,
                                stop=False,
                            )
                            nc.tensor.matmul(
                                yp[:wb, :],
                                ones_t[0:1, :wb],
                                b1_t[:],
                                start=False,
                                stop=True,
                            )
                            ys = stpool.tile([P, f2], F32, tag="ys", name="ys")
                            nc.vector.tensor_copy(ys[:wb, :], yp[:wb, :])
                            nc.sync.dma_start(
                                out=y_d[b * P : b * P + wb, :], in_=ys[:wb, :]
                            )
                        else:
                            # h2 rows = dinv * sigmoid(dinv * pb), bf16; also
                            # kept in hpre1 as the layer-1 self-loop term
                            sg = stpool.tile([P, fout], F32, tag="sg", name="sg")
                            nc.scalar.activation(
                                sg[:],
                                pb[:],
                                AF.Sigmoid,
                                scale=dinv_t[:, b : b + 1],
                            )
                            hs2 = hpre_out[:, b * f1 : (b + 1) * f1]
                            nc.vector.tensor_tensor(
                                out=hs2,
                                in0=sg[:],
                                in1=dinv_t[:, b : b + 1].to_broadcast([P, f1]),
                                op=mybir.AluOpType.mult,
                            )
                            loc_dma(h2_locA, h2_locB, hs2, b, wb)
                            if b == nblkA - 1:
                                # chunk A moves under the rest of the scatter
                                ag(h2_locA, h2_fullA)

            # ---- layer 0 ----
            gemm_layer(xT_t, w0_t, f1, hpre0_t)
            scatter_layer(h1_fullA, h1_fullB, b0_t, hpre0_t, hpre1_t,
                          is_last=False)

            # ---- layer 1 ----
            ag(h2_locB, h2_fullB)  # chunk A was emitted mid-scatter
            scatter_layer(h2_fullA, h2_fullB, b1_t, hpre1_t, None,
                          is_last=True)

    nc.compile()
    return nc


def _make_in_maps(x, W0, b0, W1, b1, plan, per_core):
    import ml_dtypes

    npc = plan.npc
    x = np.asarray(x, dtype=np.float32)
    shared = dict(
        W0=np.asarray(W0, np.float32).reshape(W0.shape[0], -1),
        W1=np.asarray(W1, np.float32).reshape(W1.shape[0], -1),
        b0=np.asarray(b0, np.float32).reshape(1, -1).astype(ml_dtypes.bfloat16),
        b1=np.asarray(b1, np.float32).reshape(1, -1),
        iota=np.tile(
            np.arange(P, dtype=np.float32)[None, :], (P, 12)
        ).astype(ml_dtypes.bfloat16),
        ident=np.eye(P, dtype=np.float32),
        identb=np.eye(P, dtype=np.float32).astype(ml_dtypes.bfloat16),
    )
    in_maps = []
    for c in range(plan.n_cores):
        m = dict(shared)
        m["xT"] = np.ascontiguousarray(x[c * npc : (c + 1) * npc, :].T)
        m["idx16"] = per_core[c]["idx16"]
        m["widx"] = np.zeros((P, 8), np.int16)
        m["slots"] = per_core[c]["slots"].astype(ml_dtypes.bfloat16)
        m["dinv"] = per_core[c]["dinv"]
        m["sqrow"] = per_core[c]["sqrow"].astype(ml_dtypes.bfloat16)
        in_maps.append(m)
    return in_maps


_CACHE = {}


def build(x, edges, W0, b0, W1, b1, n_nodes=N_NODES, n_cores=N_CORES,
          gb=GROUP_BLOCKS):
    """Returns (nc, in_maps, plan). Cached on the edge structure size."""
    plan, per_core = _build_metadata(edges, n_nodes, n_cores, gb)
    key = (n_nodes, n_cores, gb, tuple(plan.SZ.reshape(-1).tolist()))
    if key not in _CACHE:
        _CACHE[key] = _build_nc(plan, x.shape[1], W0.shape[1], W1.shape[1])
    nc = _CACHE[key]
    in_maps = _make_in_maps(x, W0, b0, W1, b1, plan, per_core)
    return nc, in_maps, plan


def kernel(x, edges, W0, b0, W1, b1):
    x = np.asarray(x)
    nc, in_maps, plan = build(x, edges, W0, b0, W1, b1)
    res = run_bass_kernel_spmd(nc, in_maps, list(range(plan.n_cores)))
    y = np.concatenate([r["y"] for r in res.results], axis=0)
    return y.astype(np.float32)
